# revision 6
# baseline (speedup 1.0000x reference)
"""Banded multi-head attention for Trainium2, sharded over 8 NeuronCores.

Problem: MHA with a |i-j| <= 128 band mask, B=2, S=4096, D=512, H=8, d_k=64.
Returns (out [2,4096,512], attn [2,8,4096,4096]).

Key structural fact: softmax of the -1e9-masked scores underflows to exactly
0.0 outside the band, so attn is zero outside a 257-wide diagonal band. The
device computes only a [128 x 384] rectangle per (head, 128-row block) - the
union of the bands of those 128 rows - and the host scatters the rectangles
into a zero-initialized full attn tensor. Everything outside the rectangles
is exactly 0 in the reference as well.

Sharding: 8 cores x 1024 contiguous query rows (core c -> batch c//4,
seq rows [(c%4)*1024, +1024)). Each core receives pre-transposed activations
(channels on partitions) plus K/V rows padded by 128 on each side so the
program is identical across cores (SPMD); sequence-edge handling is folded
into per-core additive masks prepared on the host.
"""

import sys
import numpy as np

sys.path.insert(0, "/opt/trn_rl_repo")

B, S, D, H, DK = 2, 4096, 512, 8, 64
NC = 8            # cores
RPC = S * B // NC  # 1024 query rows per core
BR = 128          # rows per block
NB = RPC // BR    # 8 blocks per core
WIN = 3 * BR      # 384: score window per block
PAD = 128         # kv pad on each side
KVR = RPC + 2 * PAD  # 1280 kv rows per core
MASKVAL = -30000.0
_CACHE = {}


def _build_program():
    import concourse.tile as tile
    from concourse import bacc, mybir

    f32 = mybir.dt.float32
    nc = bacc.Bacc("TRN2", target_bir_lowering=False, debug=False, num_devices=NC)

    # ---- per-core DRAM I/O -------------------------------------------------
    xqT = nc.dram_tensor("xqT", [D, RPC], f32, kind="ExternalInput").ap()
    xkT = nc.dram_tensor("xkT", [D, KVR], f32, kind="ExternalInput").ap()
    xvT = nc.dram_tensor("xvT", [D, KVR], f32, kind="ExternalInput").ap()
    wqT = nc.dram_tensor("wqT", [D, D], f32, kind="ExternalInput").ap()
    wkT = nc.dram_tensor("wkT", [D, D], f32, kind="ExternalInput").ap()
    wvT = nc.dram_tensor("wvT", [D, D], f32, kind="ExternalInput").ap()
    woT = nc.dram_tensor("woT", [D, D], f32, kind="ExternalInput").ap()
    bqc = nc.dram_tensor("bqc", [BR, 4], f32, kind="ExternalInput").ap()
    bkc = nc.dram_tensor("bkc", [BR, 4], f32, kind="ExternalInput").ap()
    bvb = nc.dram_tensor("bvb", [BR, D], f32, kind="ExternalInput").ap()
    bob = nc.dram_tensor("bob", [BR, D], f32, kind="ExternalInput").ap()
    maskd = nc.dram_tensor("maskd", [NB, BR, WIN], f32, kind="ExternalInput").ap()
    identd = nc.dram_tensor("identd", [BR, BR], f32, kind="ExternalInput").ap()
    attn_rect = nc.dram_tensor(
        "attn_rect", [H, NB, BR, WIN], f32, kind="ExternalOutput"
    ).ap()
    outp = nc.dram_tensor("outp", [RPC, D], f32, kind="ExternalOutput").ap()

    KC = D // BR  # 4 contraction chunks of 128

    def copy_via(use_scalar, out, in_):
        if use_scalar:
            nc.scalar.copy(out, in_)
        else:
            nc.vector.tensor_copy(out, in_)

    with tile.TileContext(nc) as tc:
        with (
            tc.tile_pool(name="persist", bufs=1) as pp,
            tc.tile_pool(name="work", bufs=3) as wp,
            tc.tile_pool(name="stat", bufs=4) as sp,
            tc.tile_pool(name="fin", bufs=2) as fp,
            tc.tile_pool(name="ps_s", bufs=2, space="PSUM") as ps_s,
            tc.tile_pool(name="ps_t", bufs=2, space="PSUM") as ps_t,
            tc.tile_pool(name="ps_o", bufs=2, space="PSUM") as ps_o,
            tc.tile_pool(name="ps_f", bufs=2, space="PSUM") as ps_f,
        ):
            # ---- load constants -------------------------------------------
            ident = pp.tile([BR, BR], f32, tag="ident")
            nc.sync.dma_start(out=ident, in_=identd)
            mask_sb = pp.tile([BR, NB, WIN], f32, tag="mask")
            nc.sync.dma_start(out=mask_sb, in_=maskd.rearrange("i p w -> p i w"))
            bq_sb = pp.tile([BR, 4], f32, tag="bq")
            nc.sync.dma_start(out=bq_sb, in_=bqc)
            bk_sb = pp.tile([BR, 4], f32, tag="bk")
            nc.sync.dma_start(out=bk_sb, in_=bkc)
            bv_sb = pp.tile([BR, D], f32, tag="bv")
            nc.sync.dma_start(out=bv_sb, in_=bvb)
            bo_sb = pp.tile([BR, D], f32, tag="bo")
            nc.sync.dma_start(out=bo_sb, in_=bob)

            w_sb = {}
            for nm, dram in (("q", wqT), ("k", wkT), ("v", wvT), ("o", woT)):
                for k in range(KC):
                    t = pp.tile([BR, D], f32, tag=f"w{nm}{k}")
                    nc.sync.dma_start(out=t, in_=dram[k * BR : (k + 1) * BR, :])
                    w_sb[nm, k] = t

            xqT_sb = []
            xkT_sb = []
            xvT_sb = []
            for k in range(KC):
                t = pp.tile([BR, RPC], f32, tag=f"xq{k}")
                nc.sync.dma_start(out=t, in_=xqT[k * BR : (k + 1) * BR, :])
                xqT_sb.append(t)
                t = pp.tile([BR, KVR], f32, tag=f"xk{k}")
                nc.sync.dma_start(out=t, in_=xkT[k * BR : (k + 1) * BR, :])
                xkT_sb.append(t)
                t = pp.tile([BR, KVR], f32, tag=f"xv{k}")
                nc.sync.dma_start(out=t, in_=xvT[k * BR : (k + 1) * BR, :])
                xvT_sb.append(t)

            # ---- projections ----------------------------------------------
            # qT[cout, row], kT[cout, row] (channels on partitions) and
            # v[row, cout] (rows on partitions).
            qT_sb = [
                pp.tile([BR, RPC], f32, tag=f"qT{m}", name=f"qT{m}") for m in range(KC)
            ]
            kT_sb = [
                pp.tile([BR, KVR], f32, tag=f"kT{m}", name=f"kT{m}") for m in range(KC)
            ]
            v_sb = [
                pp.tile([BR, D], f32, tag=f"v{r}", name=f"v{r}")
                for r in range(KVR // BR)
            ]

            for m in range(KC):  # output-channel chunk
                for n0 in range(0, RPC, 512):
                    ps = ps_f.tile([BR, 512], f32, tag="psf")
                    for k in range(KC):
                        nc.tensor.matmul(
                            ps,
                            lhsT=w_sb["q", k][:, m * BR : (m + 1) * BR],
                            rhs=xqT_sb[k][:, n0 : n0 + 512],
                            start=(k == 0),
                            stop=(k == KC - 1),
                        )
                    nc.scalar.activation(
                        out=qT_sb[m][:, n0 : n0 + 512],
                        in_=ps,
                        func=mybir.ActivationFunctionType.Identity,
                        bias=bq_sb[:, m : m + 1],
                        scale=1.0,
                    )
                for n0 in range(0, KVR, 512):
                    nn = min(512, KVR - n0)
                    ps = ps_f.tile([BR, 512], f32, tag="psf")
                    for k in range(KC):
                        nc.tensor.matmul(
                            ps[:, :nn],
                            lhsT=w_sb["k", k][:, m * BR : (m + 1) * BR],
                            rhs=xkT_sb[k][:, n0 : n0 + nn],
                            start=(k == 0),
                            stop=(k == KC - 1),
                        )
                    nc.scalar.activation(
                        out=kT_sb[m][:, n0 : n0 + nn],
                        in_=ps[:, :nn],
                        func=mybir.ActivationFunctionType.Identity,
                        bias=bk_sb[:, m : m + 1],
                        scale=1.0,
                    )
            for r in range(KVR // BR):
                ps = ps_f.tile([BR, 512], f32, tag="psf")
                for k in range(KC):
                    nc.tensor.matmul(
                        ps,
                        lhsT=xvT_sb[k][:, r * BR : (r + 1) * BR],
                        rhs=w_sb["v", k],
                        start=(k == 0),
                        stop=(k == KC - 1),
                    )
                nc.vector.tensor_add(v_sb[r], ps, bv_sb)

            # ---- attention blocks -----------------------------------------
            for i in range(NB):
                osb = fp.tile([BR, D], f32, tag="osb")  # concat of head outputs
                for h in range(H):
                    m, p0 = h // 2, (h % 2) * DK
                    q_l = qT_sb[m][p0 : p0 + DK, i * BR : (i + 1) * BR]
                    k_r = kT_sb[m][p0 : p0 + DK, i * BR : i * BR + WIN]

                    s_ps = ps_s.tile([BR, WIN], f32, tag="s")
                    # preload additive mask into PSUM via identity matmul,
                    # then accumulate the scores on top
                    nc.tensor.matmul(
                        s_ps, lhsT=ident, rhs=mask_sb[:, i, :], start=True, stop=False
                    )
                    nc.tensor.matmul(s_ps, lhsT=q_l, rhs=k_r, start=False, stop=True)

                    nmax = sp.tile([BR, 1], f32, tag="nmax")
                    nc.vector.reduce_max(
                        nmax, s_ps, axis=mybir.AxisListType.X, negate=True
                    )
                    p_sb = wp.tile([BR, WIN], f32, tag="p")
                    rsum = sp.tile([BR, 1], f32, tag="rsum")
                    nc.scalar.activation(
                        out=p_sb,
                        in_=s_ps,
                        func=mybir.ActivationFunctionType.Exp,
                        bias=nmax[:, 0:1],
                        scale=1.0,
                        accum_out=rsum[:, 0:1],
                    )
                    rinv = sp.tile([BR, 1], f32, tag="rinv")
                    nc.vector.reciprocal(rinv, rsum)
                    attn_sb = wp.tile([BR, WIN], f32, tag="attn")
                    nc.vector.tensor_scalar_mul(attn_sb, p_sb, rinv[:, 0:1])
                    nc.sync.dma_start(out=attn_rect[h, i], in_=attn_sb)

                    # transpose attn (3 x 128x128) for the PV matmul
                    aT = wp.tile([BR, WIN], f32, tag="aT")
                    for c in range(3):
                        t_ps = ps_t.tile([BR, BR], f32, tag="t")
                        nc.tensor.transpose(
                            t_ps, attn_sb[:, c * BR : (c + 1) * BR], ident
                        )
                        copy_via(c % 2 == 0, aT[:, c * BR : (c + 1) * BR], t_ps)

                    o_ps = ps_o.tile([BR, DK], f32, tag="o")
                    for c in range(3):
                        nc.tensor.matmul(
                            o_ps,
                            lhsT=aT[:, c * BR : (c + 1) * BR],
                            rhs=v_sb[i + c][:, h * DK : (h + 1) * DK],
                            start=(c == 0),
                            stop=(c == 2),
                        )
                    copy_via(h % 2 == 0, osb[:, h * DK : (h + 1) * DK], o_ps)

                # ---- output projection for this block ---------------------
                oT = fp.tile([BR, D], f32, tag="oT")
                for c in range(KC):
                    t_ps = ps_t.tile([BR, BR], f32, tag="t")
                    nc.tensor.transpose(t_ps, osb[:, c * BR : (c + 1) * BR], ident)
                    copy_via(c % 2 == 0, oT[:, c * BR : (c + 1) * BR], t_ps)
                f_ps = ps_f.tile([BR, D], f32, tag="psf")
                for k in range(KC):
                    nc.tensor.matmul(
                        f_ps,
                        lhsT=oT[:, k * BR : (k + 1) * BR],
                        rhs=w_sb["o", k],
                        start=(k == 0),
                        stop=(k == KC - 1),
                    )
                out_sb = fp.tile([BR, D], f32, tag="out")
                nc.vector.tensor_add(out_sb, f_ps, bo_sb)
                nc.sync.dma_start(out=outp[i * BR : (i + 1) * BR, :], in_=out_sb)

    nc.compile()
    return nc


def _get_program():
    if "nc" not in _CACHE:
        _CACHE["nc"] = _build_program()
    return _CACHE["nc"]


def _host_inputs(query, key, value, Wq, bq, Wk, bk, Wv, bv, Wo, bo):
    """Build the 8 per-core input maps."""
    inv = 1.0 / np.sqrt(DK)
    wqT = np.ascontiguousarray(Wq.T * inv, np.float32)
    wkT = np.ascontiguousarray(Wk.T, np.float32)
    wvT = np.ascontiguousarray(Wv.T, np.float32)
    woT = np.ascontiguousarray(Wo.T, np.float32)
    bqc = np.ascontiguousarray((bq * inv).reshape(4, BR).T, np.float32)
    bkc = np.ascontiguousarray(bk.reshape(4, BR).T, np.float32)
    bvb = np.ascontiguousarray(np.broadcast_to(bv, (BR, D)), np.float32)
    bob = np.ascontiguousarray(np.broadcast_to(bo, (BR, D)), np.float32)
    identd = np.eye(BR, dtype=np.float32)

    in_maps = []
    for c in range(NC):
        b, q0 = c // (NC // B), (c % (NC // B)) * RPC
        xqT = np.ascontiguousarray(query[b, q0 : q0 + RPC, :].T, np.float32)
        xkT = np.zeros((D, KVR), np.float32)
        xvT = np.zeros((D, KVR), np.float32)
        lo, hi = q0 - PAD, q0 + RPC + PAD
        klo, khi = max(0, lo), min(S, hi)
        xkT[:, klo - lo : khi - lo] = key[b, klo:khi, :].T
        xvT[:, klo - lo : khi - lo] = value[b, klo:khi, :].T

        mask = np.full((NB, BR, WIN), MASKVAL, np.float32)
        r = np.arange(BR)[:, None]
        w = np.arange(WIN)[None, :]
        for i in range(NB):
            jstart = q0 + i * BR - PAD
            valid = (w - r >= 0) & (w - r <= 2 * PAD) & (jstart + w >= 0) & (jstart + w < S)
            mask[i][valid] = 0.0

        in_maps.append(
            dict(
                xqT=xqT, xkT=xkT, xvT=xvT,
                wqT=wqT, wkT=wkT, wvT=wvT, woT=woT,
                bqc=bqc, bkc=bkc, bvb=bvb, bob=bob,
                maskd=mask, identd=identd,
            )
        )
    return in_maps


def kernel(query, key, value, Wq, bq, Wk, bk, Wv, bv, Wo, bo):
    query = np.asarray(query, np.float32)
    key = np.asarray(key, np.float32)
    value = np.asarray(value, np.float32)
    args = [np.asarray(a, np.float32) for a in (Wq, bq, Wk, bk, Wv, bv, Wo, bo)]

    from concourse.bass_utils import run_bass_kernel_spmd

    nc = _get_program()
    in_maps = _host_inputs(query, key, value, *args)
    res = run_bass_kernel_spmd(nc, in_maps, core_ids=list(range(NC)))

    out = np.empty((B, S, D), np.float32)
    attn = np.zeros((B, H, S, S), np.float32)
    for c in range(NC):
        b, q0 = c // (NC // B), (c % (NC // B)) * RPC
        out[b, q0 : q0 + RPC, :] = res.results[c]["outp"]
        rect = res.results[c]["attn_rect"]  # [H, NB, BR, WIN]
        for i in range(NB):
            jstart = q0 + i * BR - PAD
            wlo = max(0, -jstart)
            whi = min(WIN, S - jstart)
            attn[b, :, q0 + i * BR : q0 + (i + 1) * BR, jstart + wlo : jstart + whi] = (
                rect[:, i, :, wlo:whi]
            )
    return out, attn
